# revision 1
# baseline (speedup 1.0000x reference)
"""BboxLoss (CIoU + DFL) Trainium2 kernel.

Data-parallel over 8 NeuronCores: B=32 sharded 4 batches/core.
Per-core rows N = 4*8400 = 33600, padded to Np = 33792 = 128*264.
Row layout on device: row r <-> (partition p, free f) with r = p*264 + f.

Inputs per core:
  planes  (11, 128, 264) f32 : pbx1 pby1 pbx2 pby2 tbx1 tby1 tbx2 tby2 apx apy fg
  scores  (128, 264, 80) bf16
  pdist   (128, 264, 64) bf16
Output per core: (1, 4) f32 = [loss_iou_numerator, dfl_numerator, num_fg, 0]

Host combines partials:
  loss_iou = sum(liou) / target_scores_sum
  loss_dfl = sum(dfl) / max(4*num_fg, 1)
"""

import math

import numpy as np
import ml_dtypes

B, A, NCLS, RM = 32, 8400, 80, 16
NCORES = 8
BL = B // NCORES          # 4 batches per core
N = BL * A                # 33600 rows per core
P = 128
F = 264                   # free length; Np = P*F
NP = P * F                # 33792 (192 padded rows)
NCHUNK = 4
FC = F // NCHUNK          # 66
EPS = 1e-7
CLIP_HI = float(RM - 1) - 0.01
CV = 4.0 / math.pi**2

_BF16 = ml_dtypes.bfloat16

_cache = {}


def _build_module(nrep=1):
    import contextlib
    import concourse.bacc as bacc
    import concourse.tile as tile
    import concourse.mybir as mybir
    import concourse.bass as bass

    f32 = mybir.dt.float32
    bf16 = mybir.dt.bfloat16
    Op = mybir.AluOpType
    Act = mybir.ActivationFunctionType

    nc = bacc.Bacc("TRN2", target_bir_lowering=False, debug=False)
    planes_d = nc.dram_tensor("planes", [11, P, F], f32, kind="ExternalInput").ap()
    scores_d = nc.dram_tensor("scores", [P, F, NCLS], bf16, kind="ExternalInput").ap()
    pdist_d = nc.dram_tensor("pdist", [P, F, 4 * RM], bf16, kind="ExternalInput").ap()
    out_d = nc.dram_tensor("out4", [1, 4], f32, kind="ExternalOutput").ap()

    def bcast(ap, n):
        """Append a broadcast (step 0) innermost dim of extent n."""
        return bass.AP(tensor=ap.tensor, offset=ap.offset, ap=ap.ap + [[0, n]])

    with tile.TileContext(nc) as tc:
        with (
            tc.tile_pool(name="const", bufs=1) as const,
            tc.tile_pool(name="work", bufs=1) as work,
            tc.tile_pool(name="dma", bufs=2) as dpool,
            tc.tile_pool(name="trees", bufs=1) as tpool,
            tc.tile_pool(name="ps", bufs=1, space="PSUM") as pspool,
        ):
          with (tc.For_i(0, nrep, 1) if nrep > 1 else contextlib.nullcontext()):
              planes = const.tile([P, 11, F], f32)
              nc.sync.dma_start(out=planes, in_=planes_d.rearrange("c p f -> p c f"))
              pl = lambda i: planes[:, i, :]
              pbx1, pby1, pbx2, pby2 = pl(0), pl(1), pl(2), pl(3)
              tbx1, tby1, tbx2, tby2 = pl(4), pl(5), pl(6), pl(7)
              apx, apy, fgp = pl(8), pl(9), pl(10)

              iot = const.tile([P, 64], f32)
              nc.gpsimd.iota(
                  iot,
                  pattern=[[0, 4], [1, 16]],
                  base=0,
                  channel_multiplier=0,
                  allow_small_or_imprecise_dtypes=True,
              )
              onescol = const.tile([P, 1], f32)
              nc.vector.memset(onescol, 1.0)

              t4 = const.tile([P, F, 4], f32)
              weightp = const.tile([P, F], f32)
              sume = const.tile([P, F, 4], f32)
              hdp = const.tile([P, F, 4], f32)
              parts = const.tile([P, 4], f32)

              TT = nc.vector.tensor_tensor
              TS = nc.vector.tensor_scalar
              STT = nc.vector.scalar_tensor_tensor

              def wtile(name, shape=(P, F), dtype=f32):
                  return work.tile(list(shape), dtype, tag=name, name=name)

              # ---- t4: clipped ltrb distances (anchor vs target box) ----
              for k, (a, b) in enumerate(
                  [(apx, tbx1), (apy, tby1), (tbx2, apx), (tby2, apy)]
              ):
                  tmp = wtile("t4tmp")
                  TT(out=tmp, in0=a, in1=b, op=Op.subtract)
                  TS(
                      out=t4[:, :, k],
                      in0=tmp,
                      scalar1=0.0,
                      scalar2=CLIP_HI,
                      op0=Op.max,
                      op1=Op.min,
                  )

              # ---- CIoU on whole-core planes ----
              mn_x = wtile("mn_x"); TT(out=mn_x, in0=pbx2, in1=tbx2, op=Op.min)
              mx_x = wtile("mx_x"); TT(out=mx_x, in0=pbx1, in1=tbx1, op=Op.max)
              iw = wtile("iw")
              STT(out=iw, in0=mx_x, scalar=-1.0, in1=mn_x, op0=Op.mult, op1=Op.add)
              TS(out=iw, in0=iw, scalar1=0.0, scalar2=None, op0=Op.max)
              mn_y = work.tile([P, F], f32, tag="mn_x", name="mn_y"); TT(out=mn_y, in0=pby2, in1=tby2, op=Op.min)
              mx_y = work.tile([P, F], f32, tag="mx_x", name="mx_y"); TT(out=mx_y, in0=pby1, in1=tby1, op=Op.max)
              ih = wtile("ih")
              STT(out=ih, in0=mx_y, scalar=-1.0, in1=mn_y, op0=Op.mult, op1=Op.add)
              TS(out=ih, in0=ih, scalar1=0.0, scalar2=None, op0=Op.max)
              inter = wtile("inter"); TT(out=inter, in0=iw, in1=ih, op=Op.mult)

              w1 = wtile("w1"); TT(out=w1, in0=pbx2, in1=pbx1, op=Op.subtract)
              h1 = wtile("h1"); TT(out=h1, in0=pby2, in1=pby1, op=Op.subtract)
              w2 = wtile("w2"); TT(out=w2, in0=tbx2, in1=tbx1, op=Op.subtract)
              h2 = wtile("h2"); TT(out=h2, in0=tby2, in1=tby1, op=Op.subtract)
              a1 = wtile("a1"); TT(out=a1, in0=w1, in1=h1, op=Op.mult)
              a2 = wtile("a2"); TT(out=a2, in0=w2, in1=h2, op=Op.mult)
              s12 = wtile("s12")
              STT(out=s12, in0=a1, scalar=EPS, in1=a2, op0=Op.add, op1=Op.add)
              union = wtile("union")
              STT(out=union, in0=inter, scalar=-1.0, in1=s12, op0=Op.mult, op1=Op.add)
              runion = wtile("runion")
              nc.vector.reciprocal_approx_fast(out=runion, in_=union)
              iou = wtile("iou"); TT(out=iou, in0=inter, in1=runion, op=Op.mult)

              mxx = wtile("mxx"); TT(out=mxx, in0=pbx2, in1=tbx2, op=Op.max)
              mnx = wtile("mnx"); TT(out=mnx, in0=pbx1, in1=tbx1, op=Op.min)
              cw = wtile("cw")
              STT(out=cw, in0=mnx, scalar=-1.0, in1=mxx, op0=Op.mult, op1=Op.add)
              mxy = work.tile([P, F], f32, tag="mxx", name="mxy"); TT(out=mxy, in0=pby2, in1=tby2, op=Op.max)
              mny = work.tile([P, F], f32, tag="mnx", name="mny"); TT(out=mny, in0=pby1, in1=tby1, op=Op.min)
              ch = wtile("ch")
              STT(out=ch, in0=mny, scalar=-1.0, in1=mxy, op0=Op.mult, op1=Op.add)
              cw2 = wtile("cw2"); nc.scalar.square(out=cw2, in_=cw)
              ch2 = wtile("ch2"); nc.scalar.square(out=ch2, in_=ch)
              c2 = wtile("c2")
              STT(out=c2, in0=cw2, scalar=EPS, in1=ch2, op0=Op.add, op1=Op.add)
              rc2 = wtile("rc2")
              nc.vector.reciprocal_approx_fast(out=rc2, in_=c2)

              ux = wtile("ux"); TT(out=ux, in0=pbx1, in1=tbx1, op=Op.subtract)
              vx = wtile("vx"); TT(out=vx, in0=pbx2, in1=tbx2, op=Op.subtract)
              dx = wtile("dx"); TT(out=dx, in0=ux, in1=vx, op=Op.add)
              dx2 = wtile("dx2"); nc.scalar.square(out=dx2, in_=dx)
              uy = work.tile([P, F], f32, tag="ux", name="uy"); TT(out=uy, in0=pby1, in1=tby1, op=Op.subtract)
              vy = work.tile([P, F], f32, tag="vx", name="vy"); TT(out=vy, in0=pby2, in1=tby2, op=Op.subtract)
              dy = wtile("dy"); TT(out=dy, in0=uy, in1=vy, op=Op.add)
              dy2 = wtile("dy2"); nc.scalar.square(out=dy2, in_=dy)
              rho = wtile("rho"); TT(out=rho, in0=dx2, in1=dy2, op=Op.add)
              rr = wtile("rr"); TT(out=rr, in0=rho, in1=rc2, op=Op.mult)


              # atan(w/h) with LUT domain [-pi/2, pi/2]:
              #   m = min(w,h)/(max(w,h)+eps) in [0,1]
              #   atan(w/h) = s*pi/2 + (1-2s)*atan(m),  s = (w > h)
              # Emitted after the chunk loop so the ACT stream groups all
              # exp/abs (one table set) before the single switch to arctan.
              def atan_ratio(w, h, tag):
                  mn = wtile("atmn"); TT(out=mn, in0=w, in1=h, op=Op.min)
                  mx = wtile("atmx"); TT(out=mx, in0=w, in1=h, op=Op.max)
                  TS(out=mx, in0=mx, scalar1=EPS, scalar2=None, op0=Op.add)
                  rmx = wtile("atrm")
                  nc.vector.reciprocal_approx_fast(out=rmx, in_=mx)
                  m = wtile("atm"); TT(out=m, in0=mn, in1=rmx, op=Op.mult)
                  at = wtile(f"atv{tag}")
                  nc.scalar.activation(out=at, in_=m, func=Act.Arctan)
                  s = wtile("ats"); TT(out=s, in0=w, in1=h, op=Op.is_gt)
                  u = wtile("atu")
                  TS(out=u, in0=s, scalar1=-2.0, scalar2=1.0, op0=Op.mult, op1=Op.add)
                  atm = wtile("atw"); TT(out=atm, in0=u, in1=at, op=Op.mult)
                  res = wtile(f"atr{tag}")
                  STT(out=res, in0=s, scalar=math.pi / 2, in1=atm, op0=Op.mult, op1=Op.add)
                  return res


              # ---- streamed chunks: class-weight + DFL ----
              for c in range(NCHUNK):
                  sl = slice(c * FC, (c + 1) * FC)
                  sc = dpool.tile([P, FC, NCLS], bf16, tag="sc")
                  nc.sync.dma_start(out=sc, in_=scores_d[:, sl, :])
                  pdc = dpool.tile([P, FC, 4, RM], bf16, tag="pd")
                  nc.sync.dma_start(
                      out=pdc, in_=pdist_d[:, sl, :].rearrange("p f (k r) -> p f k r", r=RM)
                  )

                  # class-weight pairwise tree (bf16, 2x mode)
                  w40 = tpool.tile([P, FC, 40], bf16, tag="w40")
                  TT(out=w40, in0=sc[:, :, 0:40], in1=sc[:, :, 40:80], op=Op.add)
                  w20 = tpool.tile([P, FC, 20], bf16, tag="w20")
                  TT(out=w20, in0=w40[:, :, 0:20], in1=w40[:, :, 20:40], op=Op.add)
                  w10 = tpool.tile([P, FC, 10], bf16, tag="w10")
                  TT(out=w10, in0=w20[:, :, 0:10], in1=w20[:, :, 10:20], op=Op.add)
                  w5 = tpool.tile([P, FC, 5], bf16, tag="w5")
                  TT(out=w5, in0=w10[:, :, 0:5], in1=w10[:, :, 5:10], op=Op.add)
                  w2t = tpool.tile([P, FC, 2], bf16, tag="w2t")
                  TT(out=w2t, in0=w5[:, :, 0:2], in1=w5[:, :, 2:4], op=Op.add)
                  w1t = tpool.tile([P, FC], bf16, tag="w1t")
                  TT(out=w1t, in0=w2t[:, :, 0], in1=w2t[:, :, 1], op=Op.add)
                  STT(
                      out=weightp[:, sl],
                      in0=w1t,
                      scalar=0.0,
                      in1=w5[:, :, 4],
                      op0=Op.add,
                      op1=Op.add,
                  )

                  # exp
                  ec = tpool.tile([P, FC, 4, RM], bf16, tag="ec")
                  nc.scalar.activation(out=ec, in_=pdc, func=Act.Exp)
                  # sum over 16 bins: pairwise tree
                  s8 = tpool.tile([P, FC, 4, 8], bf16, tag="s8")
                  TT(out=s8, in0=ec[:, :, :, 0:8], in1=ec[:, :, :, 8:16], op=Op.add)
                  s4 = tpool.tile([P, FC, 4, 4], bf16, tag="s4")
                  TT(out=s4, in0=s8[:, :, :, 0:4], in1=s8[:, :, :, 4:8], op=Op.add)
                  s2 = tpool.tile([P, FC, 4, 2], bf16, tag="s2")
                  TT(out=s2, in0=s4[:, :, :, 0:2], in1=s4[:, :, :, 2:4], op=Op.add)
                  TT(
                      out=sume[:, sl, :],
                      in0=s2[:, :, :, 0],
                      in1=s2[:, :, :, 1],
                      op=Op.add,
                  )

                  # d = t - j  (tent distances), bf16
                  dc = tpool.tile([P, FC, 4, RM], bf16, tag="dc", bufs=2)
                  t4b = bcast(t4[:, sl, :], RM)
                  iob = bass.AP(
                      tensor=iot.tensor,
                      offset=iot.offset,
                      ap=[iot.ap[0], [0, FC], [16, 4], [1, 16]],
                  )
                  nc.gpsimd.tensor_tensor(out=dc, in0=t4b, in1=iob, op=Op.subtract)
                  # hat = relu(1 - |d|), both stages on ScalarE
                  aa = tpool.tile([P, FC, 4, RM], bf16, tag="aa", bufs=2)
                  nc.scalar.activation(out=aa, in_=dc, func=Act.Abs)
                  nh = tpool.tile([P, FC, 4, RM], bf16, tag="nh", bufs=2)
                  nc.scalar.activation(out=nh, in_=aa, func=Act.Relu, bias=1.0, scale=-1.0)
                  # hp = hat * pd ; interp = sum over bins
                  hp = tpool.tile([P, FC, 4, RM], bf16, tag="hp")
                  TT(out=hp, in0=nh, in1=pdc, op=Op.mult)
                  g8 = tpool.tile([P, FC, 4, 8], bf16, tag="g8")
                  TT(out=g8, in0=hp[:, :, :, 0:8], in1=hp[:, :, :, 8:16], op=Op.add)
                  g4 = tpool.tile([P, FC, 4, 4], bf16, tag="g4")
                  TT(out=g4, in0=g8[:, :, :, 0:4], in1=g8[:, :, :, 4:8], op=Op.add)
                  g2 = tpool.tile([P, FC, 4, 2], bf16, tag="g2")
                  TT(out=g2, in0=g4[:, :, :, 0:2], in1=g4[:, :, :, 2:4], op=Op.add)
                  TT(
                      out=hdp[:, sl, :],
                      in0=g2[:, :, :, 0],
                      in1=g2[:, :, :, 1],
                      op=Op.add,
                  )


              at1 = atan_ratio(w1, h1, "1")
              at2 = atan_ratio(w2, h2, "2")
              dat = work.tile([P, F], f32, tag="mxx", name="dat"); TT(out=dat, in0=at1, in1=at2, op=Op.subtract)
              v0 = wtile("v0"); TT(out=v0, in0=dat, in1=dat, op=Op.mult)

              da = work.tile([P, F], f32, tag="ih", name="da")
              TS(out=da, in0=v0, scalar1=CV, scalar2=1.0 + EPS, op0=Op.mult, op1=Op.add)
              den = work.tile([P, F], f32, tag="mn_x", name="den"); TT(out=den, in0=da, in1=iou, op=Op.subtract)
              rden = work.tile([P, F], f32, tag="atmn", name="rden")
              nc.vector.reciprocal_approx_fast(out=rden, in_=den)
              v2 = work.tile([P, F], f32, tag="mx_x", name="v2"); nc.scalar.square(out=v2, in_=v0)
              vr = work.tile([P, F], f32, tag="mnx", name="vr"); TT(out=vr, in0=v2, in1=rden, op=Op.mult)

              z1 = work.tile([P, F], f32, tag="ux", name="z1")
              TS(out=z1, in0=rr, scalar1=0.25, scalar2=1.0, op0=Op.mult, op1=Op.add)
              z2 = work.tile([P, F], f32, tag="vx", name="z2")
              STT(out=z2, in0=vr, scalar=CV * CV, in1=z1, op0=Op.mult, op1=Op.add)
              q = work.tile([P, F], f32, tag="atmx", name="q"); TT(out=q, in0=z2, in1=iou, op=Op.subtract)

              # ---- DFL finale ----
              nc.scalar.activation(out=sume, in_=sume, func=Act.Ln)
              dfl4 = wtile("dfl4", (P, F, 4))
              TT(out=dfl4, in0=sume, in1=hdp, op=Op.subtract)
              dflr2 = wtile("dflr2", (P, F, 2))
              TT(out=dflr2, in0=dfl4[:, :, 0:2], in1=dfl4[:, :, 2:4], op=Op.add)
              dflrow = wtile("dflrow")
              TT(out=dflrow, in0=dflr2[:, :, 0], in1=dflr2[:, :, 1], op=Op.add)

              # ---- masked accumulations ----
              Ax = mybir.AxisListType
              wq = work.tile([P, F], f32, tag="atrm", name="wq"); TT(out=wq, in0=q, in1=weightp, op=Op.mult)
              scr = wtile("scr")
              TT(out=scr, in0=wq, in1=fgp, op=Op.mult)
              nc.vector.tensor_reduce(out=parts[:, 0:1], in_=scr, axis=Ax.X, op=Op.add)
              scr2 = wtile("scr2")
              TT(out=scr2, in0=dflrow, in1=fgp, op=Op.mult)
              nc.vector.tensor_reduce(out=parts[:, 1:2], in_=scr2, axis=Ax.X, op=Op.add)
              nc.vector.tensor_reduce(out=parts[:, 2:3], in_=fgp, axis=Ax.X, op=Op.add)
              nc.vector.memset(parts[:, 3:4], 0.0)

              # ---- cross-partition reduce via TensorE ----
              psum = pspool.tile([1, 4], f32)
              nc.tensor.matmul(psum, onescol, parts, start=True, stop=True)
              outsb = const.tile([1, 4], f32)
              nc.scalar.copy(out=outsb, in_=psum)
              nc.sync.dma_start(out=out_d, in_=outsb)

    nc.compile()
    return nc


def _get_module(nrep=1):
    key = ("nc", nrep)
    if key not in _cache:
        _cache[key] = _build_module(nrep)
    return _cache[key]


def _prep_core_inputs(pred_dist, pred_bboxes, anchor_points, target_bboxes,
                      target_scores, fg_mask):
    """Shard + lay out host-side per-core inputs."""
    pdf = np.ascontiguousarray(pred_dist.reshape(B * A, 4 * RM))
    pbf = np.ascontiguousarray(pred_bboxes.reshape(B * A, 4))
    tbf = np.ascontiguousarray(target_bboxes.reshape(B * A, 4))
    tsf = np.ascontiguousarray(target_scores.reshape(B * A, NCLS))
    fgf = np.ascontiguousarray(fg_mask.reshape(B * A)).astype(np.float32)
    ap_t = np.tile(np.asarray(anchor_points, dtype=np.float32), (BL, 1))  # (N,2)

    in_maps = []
    for i in range(NCORES):
        sl = slice(i * N, (i + 1) * N)
        planes = np.zeros((11, NP), dtype=np.float32)
        pb = pbf[sl]
        tb = tbf[sl]
        for k in range(4):
            planes[k, :N] = pb[:, k]
            planes[4 + k, :N] = tb[:, k]
        planes[8, :N] = ap_t[:, 0]
        planes[9, :N] = ap_t[:, 1]
        planes[10, :N] = fgf[sl]
        scores = np.zeros((NP, NCLS), dtype=_BF16)
        scores[:N] = tsf[sl].astype(_BF16)
        pdist = np.zeros((NP, 4 * RM), dtype=_BF16)
        pdist[:N] = pdf[sl].astype(_BF16)
        in_maps.append(
            {
                "planes": planes.reshape(11, P, F),
                "scores": scores.reshape(P, F, NCLS),
                "pdist": pdist.reshape(P, F, 4 * RM),
            }
        )
    return in_maps


def kernel(pred_dist, pred_bboxes, anchor_points, target_bboxes,
           target_scores, target_scores_sum, fg_mask):
    from concourse.bass_utils import run_bass_kernel_spmd

    nc = _get_module()
    in_maps = _prep_core_inputs(
        pred_dist, pred_bboxes, anchor_points, target_bboxes, target_scores, fg_mask
    )
    res = run_bass_kernel_spmd(nc, in_maps, list(range(NCORES)))
    partials = np.stack([res.results[c]["out4"][0] for c in range(NCORES)])  # (8,4)
    liou_num = float(partials[:, 0].sum())
    dfl_num = float(partials[:, 1].sum())
    num_fg = float(partials[:, 2].sum())
    loss_iou = liou_num / float(np.asarray(target_scores_sum))
    loss_dfl = dfl_num / max(num_fg * 4.0, 1.0)
    return (np.float32(loss_iou), np.float32(loss_dfl))

